# revision 1
# baseline (speedup 1.0000x reference)
"""Trainium2 Bass kernel for LoopedMLPForLM.

Model: x_emb = token_emb[x] + pos_emb
       x_proj = x_emb @ W_in^T + b_in
       h <- tanh(x_proj + h @ W_rec^T + b_rec)   (20 steps, h0 = 0)
       logits = h @ lm_head^T + b_lm

Sharding: data-parallel over the 8192 tokens -> 1024 tokens per core on 8
NeuronCores; all weights replicated.  On-chip layout keeps activations
feature-major ([H partitions, tokens]) so the recurrence needs no
transposes and biases are per-partition; the lm_head matmul flips roles
(stationary = h, moving = lm_head^T) so logits come out [tokens, vocab].
Matmuls run in bf16 with fp32 PSUM accumulation.
"""

import sys

sys.path.insert(0, "/opt/trn_rl_repo")

from contextlib import ExitStack

import ml_dtypes
import numpy as np

import concourse.bacc as bacc
import concourse.bass as bass
import concourse.tile as tile
from concourse import mybir
from concourse.bass import IndirectOffsetOnAxis
from concourse.bass_utils import run_bass_kernel_spmd
from concourse.masks import make_identity

P = 128
NCORES = 8
BF16 = mybir.dt.bfloat16
F32 = mybir.dt.float32
I32 = mybir.dt.int32
AF = mybir.ActivationFunctionType

# Problem shape (hardcoded per contract)
B, S = 4, 2048
HID = 1024
VOCAB = 32000
STEPS = 20
TOK = (B * S) // NCORES  # tokens per core


def build_nc(tok=TOK, hid=HID, vocab=VOCAB, steps=STEPS, vb=512):
    kb = hid // P  # contraction (k) blocks
    ob = hid // P  # output-feature blocks
    tb = tok // P  # token blocks of 128
    chunk = min(512, tok)  # token chunk = one PSUM bank of fp32
    nchunk = tok // chunk

    nc = bacc.Bacc("TRN2", target_bir_lowering=False, debug=False, num_devices=NCORES)

    xi = nc.dram_tensor("xi", [tb, P, 1], I32, kind="ExternalInput")
    emb = nc.dram_tensor("emb", [vocab, hid], BF16, kind="ExternalInput")
    pos = nc.dram_tensor("pos", [tok, hid], BF16, kind="ExternalInput")
    wiT = nc.dram_tensor("wiT", [hid, hid], BF16, kind="ExternalInput")
    wrT = nc.dram_tensor("wrT", [hid, hid], BF16, kind="ExternalInput")
    btot = nc.dram_tensor("btot", [P, ob], F32, kind="ExternalInput")
    lmT = nc.dram_tensor("lmT", [hid, vocab], BF16, kind="ExternalInput")
    lmb = nc.dram_tensor("lmb", [vocab], F32, kind="ExternalInput")
    y = nc.dram_tensor("y", [tok, vocab], F32, kind="ExternalOutput")

    with tile.TileContext(nc) as tc:
        with ExitStack() as ctx:
            consts = ctx.enter_context(tc.tile_pool(name="consts", bufs=1))
            iop = ctx.enter_context(tc.tile_pool(name="iop", bufs=3))
            gp = ctx.enter_context(tc.tile_pool(name="gp", bufs=3))
            tmps = ctx.enter_context(tc.tile_pool(name="tmps", bufs=4))
            lmwp = ctx.enter_context(tc.tile_pool(name="lmwp", bufs=2))
            lmbp = ctx.enter_context(tc.tile_pool(name="lmbp", bufs=2))
            outp = ctx.enter_context(tc.tile_pool(name="outp", bufs=4))
            psum = ctx.enter_context(tc.tile_pool(name="psum", bufs=8, space="PSUM"))

            ident = consts.tile([P, P], BF16, name="ident")
            make_identity(nc, ident[:])
            btot_sb = consts.tile([P, ob], F32, name="btot_sb")
            nc.sync.dma_start(out=btot_sb[:], in_=btot.ap())
            wiT_sb = consts.tile([P, kb, hid], BF16, name="wiT_sb")
            nc.sync.dma_start(
                out=wiT_sb[:], in_=wiT.ap().rearrange("(kb p) m -> p kb m", p=P)
            )
            wrT_sb = consts.tile([P, kb, hid], BF16, name="wrT_sb")
            nc.sync.dma_start(
                out=wrT_sb[:], in_=wrT.ap().rearrange("(kb p) m -> p kb m", p=P)
            )
            # activations, feature-major: [feature partition, feature block, token]
            xT = consts.tile([P, kb, tok], BF16, name="xT")
            xb = consts.tile([P, ob, tok], F32, name="xb")
            hA = consts.tile([P, kb, tok], BF16, name="hA")
            hB = consts.tile([P, kb, tok], BF16, name="hB")

            emb_ap = emb.ap()
            pos_ap = pos.ap()
            xi_ap = xi.ap()

            # ---- embedding gather + pos add, then transpose to feature-major
            for t in range(tb):
                idx = iop.tile([P, 1], I32, name="idx")
                nc.sync.dma_start(out=idx[:], in_=xi_ap[t])
                g = gp.tile([P, hid], BF16, name="g")
                nc.gpsimd.indirect_dma_start(
                    out=g[:],
                    out_offset=None,
                    in_=emb_ap,
                    in_offset=IndirectOffsetOnAxis(ap=idx[:, :1], axis=0),
                )
                pp = gp.tile([P, hid], BF16, name="pp")
                nc.sync.dma_start(out=pp[:], in_=pos_ap[t * P : (t + 1) * P, :])
                xe = gp.tile([P, hid], BF16, name="xe")
                nc.vector.tensor_add(xe[:], g[:], pp[:])
                for k in range(kb):
                    pt = psum.tile([P, P], BF16, name="pt", tag="ps")
                    nc.tensor.transpose(
                        out=pt[:], in_=xe[:, k * P : (k + 1) * P], identity=ident[:]
                    )
                    nc.scalar.copy(out=xT[:, k, t * P : (t + 1) * P], in_=pt[:])

            # ---- x_proj = x_emb @ W_in^T, then xb = x_proj + (b_in + b_rec)
            for c in range(nchunk):
                cs = slice(c * chunk, (c + 1) * chunk)
                for o in range(ob):
                    ps = psum.tile([P, chunk], F32, name="ps", tag="ps")
                    for k in range(kb):
                        nc.tensor.matmul(
                            out=ps[:],
                            lhsT=wiT_sb[:, k, o * P : (o + 1) * P],
                            rhs=xT[:, k, cs],
                            start=(k == 0),
                            stop=(k == kb - 1),
                        )
                    nc.scalar.activation(
                        out=xb[:, o, cs],
                        in_=ps[:],
                        func=AF.Identity,
                        bias=btot_sb[:, o : o + 1],
                    )

            # ---- h1 = tanh(xb)  (h0 = 0)
            for o in range(ob):
                nc.scalar.activation(out=hA[:, o, :], in_=xb[:, o, :], func=AF.Tanh)

            # ---- recurrence: h <- tanh(xb + h @ W_rec^T), 19 more steps
            hsrc, hdst = hA, hB
            for _ in range(steps - 1):
                for c in range(nchunk):
                    cs = slice(c * chunk, (c + 1) * chunk)
                    for o in range(ob):
                        ps = psum.tile([P, chunk], F32, name="ps", tag="ps")
                        for k in range(kb):
                            nc.tensor.matmul(
                                out=ps[:],
                                lhsT=wrT_sb[:, k, o * P : (o + 1) * P],
                                rhs=hsrc[:, k, cs],
                                start=(k == 0),
                                stop=(k == kb - 1),
                            )
                        tmp = tmps.tile([P, chunk], F32, name="tmp")
                        nc.vector.tensor_add(tmp[:], ps[:], xb[:, o, cs])
                        nc.scalar.activation(
                            out=hdst[:, o, cs], in_=tmp[:], func=AF.Tanh
                        )
                hsrc, hdst = hdst, hsrc

            # ---- logits = h @ lm_head^T + b_lm   (stationary = h token block)
            lmT_r = lmT.ap().rearrange("(kb p) v -> p kb v", p=P)
            y_ap = y.ap()
            voff = 0
            while voff < vocab:
                vsz = min(vb, vocab - voff)
                wt = lmwp.tile([P, kb, vb], BF16, name="wt")
                nc.sync.dma_start(out=wt[:, :, :vsz], in_=lmT_r[:, :, voff : voff + vsz])
                bt = lmbp.tile([P, vb], F32, name="bt")
                nc.gpsimd.dma_start(
                    out=bt[:, :vsz], in_=bass.AP(lmb, voff, [[0, P], [1, vsz]])
                )
                for t in range(tb):
                    ps = psum.tile([P, vb], F32, name="ps", tag="ps")
                    for k in range(kb):
                        nc.tensor.matmul(
                            out=ps[:, :vsz],
                            lhsT=hsrc[:, k, t * P : (t + 1) * P],
                            rhs=wt[:, k, :vsz],
                            start=(k == 0),
                            stop=(k == kb - 1),
                        )
                    ot = outp.tile([P, vb], F32, name="ot")
                    nc.vector.tensor_add(ot[:, :vsz], ps[:, :vsz], bt[:, :vsz])
                    nc.sync.dma_start(
                        out=y_ap[t * P : (t + 1) * P, voff : voff + vsz],
                        in_=ot[:, :vsz],
                    )
                voff += vsz

    nc.compile()
    return nc


_NC = None


def _get_nc():
    global _NC
    if _NC is None:
        _NC = build_nc()
    return _NC


def _make_in_maps(x, token_emb, pos_emb, W_in_w, W_in_b, W_rec_w, W_rec_b, lm_head_w, lm_head_b):
    bf = ml_dtypes.bfloat16
    x_flat = np.asarray(x).astype(np.int32).reshape(-1)
    emb_b = np.asarray(token_emb, dtype=np.float32).astype(bf)
    pos_b = np.asarray(pos_emb, dtype=np.float32).astype(bf)
    wiT = np.ascontiguousarray(np.asarray(W_in_w, np.float32).T).astype(bf)
    wrT = np.ascontiguousarray(np.asarray(W_rec_w, np.float32).T).astype(bf)
    lmT = np.ascontiguousarray(np.asarray(lm_head_w, np.float32).T).astype(bf)
    btot = np.ascontiguousarray(
        (np.asarray(W_in_b, np.float32) + np.asarray(W_rec_b, np.float32))
        .reshape(HID // P, P)
        .T
    )
    lmb = np.asarray(lm_head_b, np.float32)

    in_maps = []
    for c in range(NCORES):
        toks = x_flat[c * TOK : (c + 1) * TOK]
        s0 = (c * TOK) % S
        in_maps.append(
            {
                "xi": np.ascontiguousarray(toks.reshape(TOK // P, P, 1)),
                "emb": emb_b,
                "pos": np.ascontiguousarray(pos_b[s0 : s0 + TOK]),
                "wiT": wiT,
                "wrT": wrT,
                "btot": btot,
                "lmT": lmT,
                "lmb": lmb,
            }
        )
    return in_maps


def _run(inputs: dict, trace: bool = False, **kwargs):
    nc = _get_nc()
    in_maps = _make_in_maps(**inputs)
    return run_bass_kernel_spmd(
        nc, in_maps, core_ids=list(range(NCORES)), trace=trace, **kwargs
    )


def kernel(**inputs) -> np.ndarray:
    res = _run(inputs, trace=False)
    out = np.concatenate([r["y"] for r in res.results], axis=0)
    return np.ascontiguousarray(out.reshape(B, S, VOCAB).astype(np.float32))


# revision 9
# speedup vs baseline: 54.6935x; 54.6935x over previous
"""Trainium2 Bass kernel for LoopedMLPForLM.

Model: x_emb = token_emb[x] + pos_emb
       x_proj = x_emb @ W_in^T + b_in
       h <- tanh(x_proj + h @ W_rec^T + b_rec)   (20 steps, h0 = 0)
       logits = h @ lm_head^T + b_lm

Sharding: data-parallel over the 8192 tokens -> 1024 tokens per core on 8
NeuronCores; all weights replicated.  On-chip layout keeps activations
feature-major ([H partitions, tokens]) so the recurrence needs no
transposes and biases are per-partition; the lm_head matmul flips roles
(stationary = h, moving = lm_head^T) so logits come out [tokens, vocab].
Matmuls run in bf16 with fp32 PSUM accumulation.
"""

import sys

sys.path.insert(0, "/opt/trn_rl_repo")

from contextlib import ExitStack

import ml_dtypes
import numpy as np

import concourse.bacc as bacc
import concourse.bass as bass
import concourse.tile as tile
from concourse import mybir
from concourse.bass import IndirectOffsetOnAxis
from concourse.bass_utils import run_bass_kernel_spmd
from concourse.masks import make_identity

P = 128
NCORES = 8
BF16 = mybir.dt.bfloat16
F32 = mybir.dt.float32
I32 = mybir.dt.int32
AF = mybir.ActivationFunctionType

# Problem shape (hardcoded per contract)
B, S = 4, 2048
HID = 1024
VOCAB = 32000
STEPS = 20
TOK = (B * S) // NCORES  # tokens per core


def build_nc(tok=TOK, hid=HID, vocab=VOCAB, steps=STEPS, vb=512):
    kb = hid // P  # contraction (k) blocks
    ob = hid // P  # output-feature blocks
    tb = tok // P  # token blocks of 128
    chunk = min(512, tok)  # token chunk = one PSUM bank of fp32
    nchunk = tok // chunk

    nc = bacc.Bacc(
        "TRN2",
        target_bir_lowering=False,
        debug=False,
        num_devices=NCORES,
        num_swdge_queues=4,
    )

    xi = nc.dram_tensor("xi", [tb, P, 1], I32, kind="ExternalInput")
    emb = nc.dram_tensor("emb", [vocab, hid], BF16, kind="ExternalInput")
    pos = nc.dram_tensor("pos", [tok, hid], BF16, kind="ExternalInput")
    wiT = nc.dram_tensor("wiT", [hid, hid], BF16, kind="ExternalInput")
    wrT = nc.dram_tensor("wrT", [hid, hid], BF16, kind="ExternalInput")
    btot = nc.dram_tensor("btot", [P, ob], F32, kind="ExternalInput")
    lmT = nc.dram_tensor("lmT", [hid, vocab], BF16, kind="ExternalInput")
    # lm_head bias pre-broadcast across partitions on the host
    lmbb = nc.dram_tensor("lmbb", [P, vocab], BF16, kind="ExternalInput")
    y = nc.dram_tensor("y", [tok, vocab], F32, kind="ExternalOutput")

    with tile.TileContext(nc) as tc:
        with ExitStack() as ctx:
            consts = ctx.enter_context(tc.tile_pool(name="consts", bufs=1))
            iop = ctx.enter_context(tc.tile_pool(name="iop", bufs=3))
            gp = ctx.enter_context(tc.tile_pool(name="gp", bufs=3))
            tmps = ctx.enter_context(tc.tile_pool(name="tmps", bufs=4))
            lmwp = ctx.enter_context(tc.tile_pool(name="lmwp", bufs=3))
            lmbp = ctx.enter_context(tc.tile_pool(name="lmbp", bufs=2))
            outp = ctx.enter_context(tc.tile_pool(name="outp", bufs=4))
            psum = ctx.enter_context(tc.tile_pool(name="psum", bufs=8, space="PSUM"))

            ident = consts.tile([P, P], BF16, name="ident")
            make_identity(nc, ident[:])
            # activations, feature-major: [feature partition, feature block, token]
            xT = consts.tile([P, kb, tok], BF16, name="xT")
            xb = consts.tile([P, ob, tok], F32, name="xb")
            hA = consts.tile([P, kb, tok], BF16, name="hA")
            hB = consts.tile([P, kb, tok], BF16, name="hB")

            emb_ap = emb.ap()
            pos_ap = pos.ap()
            xi_ap = xi.ap()

            # ---- embedding gather + pos add, then transpose to feature-major
            # (issued before the weight DMAs so the first transposes aren't
            # queued behind multi-MB weight transfers)
            for t in range(tb):
                idx = iop.tile([P, 1], I32, name="idx")
                nc.sync.dma_start(out=idx[:], in_=xi_ap[t])
                g = gp.tile([P, hid], BF16, name="g")
                nc.gpsimd.indirect_dma_start(
                    out=g[:],
                    out_offset=None,
                    in_=emb_ap,
                    in_offset=IndirectOffsetOnAxis(ap=idx[:, :1], axis=0),
                )
                pp = gp.tile([P, hid], BF16, name="pp")
                nc.sync.dma_start(out=pp[:], in_=pos_ap[t * P : (t + 1) * P, :])
                xe = gp.tile([P, hid], BF16, name="xe")
                nc.vector.tensor_add(xe[:], g[:], pp[:])
                for k in range(kb):
                    pt = psum.tile([P, P], BF16, name="pt", tag="ps")
                    nc.tensor.transpose(
                        out=pt[:], in_=xe[:, k * P : (k + 1) * P], identity=ident[:]
                    )
                    nc.scalar.copy(out=xT[:, k, t * P : (t + 1) * P], in_=pt[:])

            btot_sb = consts.tile([P, ob], F32, name="btot_sb")
            nc.sync.dma_start(out=btot_sb[:], in_=btot.ap())
            wiT_sb = consts.tile([P, kb, hid], BF16, name="wiT_sb")
            nc.sync.dma_start(
                out=wiT_sb[:], in_=wiT.ap().rearrange("(kb p) m -> p kb m", p=P)
            )
            wrT_sb = consts.tile([P, kb, hid], BF16, name="wrT_sb")
            nc.sync.dma_start(
                out=wrT_sb[:], in_=wrT.ap().rearrange("(kb p) m -> p kb m", p=P)
            )

            # ---- x_proj = x_emb @ W_in^T, then xb = x_proj + (b_in + b_rec)
            for c in range(nchunk):
                cs = slice(c * chunk, (c + 1) * chunk)
                for o in range(ob):
                    ps = psum.tile([P, chunk], F32, name="ps", tag="ps")
                    for k in range(kb):
                        nc.tensor.matmul(
                            out=ps[:],
                            lhsT=wiT_sb[:, k, o * P : (o + 1) * P],
                            rhs=xT[:, k, cs],
                            start=(k == 0),
                            stop=(k == kb - 1),
                        )
                    nc.scalar.activation(
                        out=xb[:, o, cs],
                        in_=ps[:],
                        func=AF.Identity,
                        bias=btot_sb[:, o : o + 1],
                    )

            # ---- h1 = tanh(xb)  (h0 = 0)
            for o in range(ob):
                nc.scalar.activation(out=hA[:, o, :], in_=xb[:, o, :], func=AF.Tanh)

            # ---- recurrence: h <- tanh(xb + h @ W_rec^T), 19 more steps
            hsrc, hdst = hA, hB
            for _ in range(steps - 1):
                for c in range(nchunk):
                    cs = slice(c * chunk, (c + 1) * chunk)
                    for o in range(ob):
                        ps = psum.tile([P, chunk], F32, name="ps", tag="ps")
                        for k in range(kb):
                            nc.tensor.matmul(
                                out=ps[:],
                                lhsT=wrT_sb[:, k, o * P : (o + 1) * P],
                                rhs=hsrc[:, k, cs],
                                start=(k == 0),
                                stop=(k == kb - 1),
                            )
                        tmp = tmps.tile([P, chunk], F32, name="tmp")
                        nc.vector.tensor_add(tmp[:], ps[:], xb[:, o, cs])
                        nc.scalar.activation(
                            out=hdst[:, o, cs], in_=tmp[:], func=AF.Tanh
                        )
                hsrc, hdst = hdst, hsrc

            # ---- logits = h @ lm_head^T + b_lm   (stationary = h token block)
            lmT_r = lmT.ap().rearrange("(kb p) v -> p kb v", p=P)
            lmbb_ap = lmbb.ap()
            y_ap = y.ap()
            voff = 0
            while voff < vocab:
                vsz = min(vb, vocab - voff)
                wt = lmwp.tile([P, kb, vb], BF16, name="wt")
                nc.sync.dma_start(out=wt[:, :, :vsz], in_=lmT_r[:, :, voff : voff + vsz])
                bt = lmbp.tile([P, vb], BF16, name="bt")
                nc.sync.dma_start(out=bt[:, :vsz], in_=lmbb_ap[:, voff : voff + vsz])
                for t in range(tb):
                    ps = psum.tile([P, vb], F32, name="ps", tag="ps")
                    for k in range(kb):
                        nc.tensor.matmul(
                            out=ps[:, :vsz],
                            lhsT=hsrc[:, k, t * P : (t + 1) * P],
                            rhs=wt[:, k, :vsz],
                            start=(k == 0),
                            stop=(k == kb - 1),
                        )
                    ot = outp.tile([P, vb], F32, name="ot")
                    nc.vector.tensor_add(ot[:, :vsz], ps[:, :vsz], bt[:, :vsz])
                    nc.sync.dma_start(
                        out=y_ap[t * P : (t + 1) * P, voff : voff + vsz],
                        in_=ot[:, :vsz],
                    )
                voff += vsz

    nc.compile()
    return nc


_NC = None


def _get_nc():
    global _NC
    if _NC is None:
        _NC = build_nc()
    return _NC


def _make_in_maps(x, token_emb, pos_emb, W_in_w, W_in_b, W_rec_w, W_rec_b, lm_head_w, lm_head_b):
    bf = ml_dtypes.bfloat16
    x_flat = np.asarray(x).astype(np.int32).reshape(-1)
    emb_b = np.asarray(token_emb, dtype=np.float32).astype(bf)
    pos_b = np.asarray(pos_emb, dtype=np.float32).astype(bf)
    wiT = np.ascontiguousarray(np.asarray(W_in_w, np.float32).T).astype(bf)
    wrT = np.ascontiguousarray(np.asarray(W_rec_w, np.float32).T).astype(bf)
    lmT = np.ascontiguousarray(np.asarray(lm_head_w, np.float32).T).astype(bf)
    btot = np.ascontiguousarray(
        (np.asarray(W_in_b, np.float32) + np.asarray(W_rec_b, np.float32))
        .reshape(HID // P, P)
        .T
    )
    lmbb = np.ascontiguousarray(
        np.broadcast_to(np.asarray(lm_head_b, np.float32).astype(bf)[None, :], (P, VOCAB))
    )

    in_maps = []
    for c in range(NCORES):
        toks = x_flat[c * TOK : (c + 1) * TOK]
        s0 = (c * TOK) % S
        in_maps.append(
            {
                "xi": np.ascontiguousarray(toks.reshape(TOK // P, P, 1)),
                "emb": emb_b,
                "pos": np.ascontiguousarray(pos_b[s0 : s0 + TOK]),
                "wiT": wiT,
                "wrT": wrT,
                "btot": btot,
                "lmT": lmT,
                "lmbb": lmbb,
            }
        )
    return in_maps


def _run(inputs: dict, trace: bool = False, **kwargs):
    nc = _get_nc()
    in_maps = _make_in_maps(**inputs)
    return run_bass_kernel_spmd(
        nc, in_maps, core_ids=list(range(NCORES)), trace=trace, **kwargs
    )


def kernel(**inputs) -> np.ndarray:
    res = _run(inputs, trace=False)
    out = np.concatenate([r["y"] for r in res.results], axis=0)
    return np.ascontiguousarray(out.reshape(B, S, VOCAB).astype(np.float32))
